# revision 32
# baseline (speedup 1.0000x reference)
"""Trainium2 Bass kernel for nn_DSVF (direct-form-I biquad IIR over time).

Algorithm
---------
The biquad is a stable IIR (poles well inside the unit circle for any
parameters setup_inputs-style params produce), so its impulse response decays
geometrically.  We truncate it at K taps (K chosen at runtime from the actual
coefficients so the truncated tail is < 1e-9; K is ~32 here) and compute the
filter as a causal FIR convolution — numerically indistinguishable from the
reference (the truncation error is far below the reference's own fp32
accumulation noise).

Kernel structure
----------------
The convolution is a Toeplitz matmul over 128-sample time chunks:

    yT[c] = H_a^T @ xT[c] + H_b^T @ xT[c-1]

where xT/yT are time-major [time, batch] chunks ([128, 512] on chip:
time-within-chunk on partitions, batch on the free axis), and
H_a[s, j] = h[j - s], H_b[s, j] = h[128 + j - s] are [128, 128] Toeplitz
pieces of the impulse response.  Each output chunk is exactly two tensor-
engine matmuls accumulating in one PSUM bank, with the moving operand at the
fp32 maximum free size (512).  No on-chip transposes: the host hands each
core its shard already in [time, batch] layout (part of the sharding step),
and un-transposes the [time, batch] result on the way back.  Matmuls run in
float32r (one half-clock PE pass instead of fp32's two).

Sharding
--------
8 cores = 4 batch shards x 2 time shards.  Batch is pure data parallel; the
time split is exact because the truncated FIR only needs a 128-sample halo,
which each second-half core receives (first-half cores get a zero halo).
The 5 scalar filter parameters are folded into H_a/H_b on the host.
"""
import math

import numpy as np

import concourse.bacc as bacc
import concourse.mybir as mybir
import concourse.tile as tile
from concourse.bass_utils import axon_active, run_bass_kernel_spmd

N_CORES = 8
BS, TH = 4, 2          # batch shards x time shards
L = 128                # time chunk (partition count)
MAX_K = L + 1          # max taps the 2-matmul window supports
PIECE = 8              # chunks per DMA piece


def _filter_coeffs(g, r, m_hp, m_bp, m_lp):
    """float32 mirror of the reference coefficient computation."""
    f32 = np.float32
    g = f32(g)
    r = f32(r)
    m_hp, m_bp, m_lp = f32(m_hp), f32(m_bp), f32(m_lp)
    gg = np.tan(f32(math.pi) * (f32(1) / (f32(1) + np.exp(-g))) / f32(2))
    rr = np.log1p(np.exp(r)).astype(f32)
    g2 = gg * gg
    b = np.array(
        [g2 * m_lp + gg * m_bp + m_hp,
         2 * g2 * m_lp - 2 * m_hp,
         g2 * m_lp - gg * m_bp + m_hp],
        dtype=f32,
    )
    a = np.array(
        [g2 + 2 * rr * gg + 1,
         2 * g2 - 2,
         g2 - 2 * rr * gg + 1],
        dtype=f32,
    )
    a = (a / a[0]).astype(f32)
    # reference divides b by the already-normalized a[0] == 1: b unchanged
    return b.astype(np.float64), a.astype(np.float64)


def _impulse_response(b, a, n):
    h = np.zeros(n, dtype=np.float64)
    for t in range(n):
        acc = b[0] if t == 0 else 0.0
        if t == 1:
            acc += b[1]
        if t == 2:
            acc += b[2]
        if t >= 1:
            acc -= a[1] * h[t - 1]
        if t >= 2:
            acc -= a[2] * h[t - 2]
        h[t] = acc
    return h


def _choose_taps(h):
    tail = np.cumsum(np.abs(h[::-1]))[::-1]
    for k in range(3, len(h)):
        if tail[k] < 1e-9:
            return k
    raise ValueError(
        f"impulse response decays too slowly for the FIR formulation "
        f"(tail at {len(h)} taps = {tail[-1]:.3e})"
    )


def _toeplitz_pieces(h, K):
    h_a = np.zeros((L, L), dtype=np.float32)
    h_b = np.zeros((L, L), dtype=np.float32)
    for s in range(L):
        for j in range(L):
            k = j - s
            if 0 <= k < K:
                h_a[s, j] = h[k]
            k = L + j - s
            if 0 <= k < K:
                h_b[s, j] = h[k]
    return h_a, h_b


def _piece_sizes(n, first_small):
    """Split n chunks into DMA pieces; small leading pieces so compute can
    start early (and small trailing output pieces so the last store is
    quick)."""
    sizes = []
    remaining = n
    for w in first_small:
        if remaining <= 0:
            break
        w = min(w, remaining)
        sizes.append(w)
        remaining -= w
    while remaining > 0:
        w = min(PIECE, remaining)
        sizes.append(w)
        remaining -= w
    return sizes


def _build_module(b_core, t_core):
    """Per-core SPMD program: FIR chunks via two stationary-H matmuls each."""
    assert b_core <= 512 and t_core % L == 0
    C = t_core // L                     # output chunks per core
    CI = C + 1                          # input chunks incl. 1 halo chunk

    in_sizes = _piece_sizes(CI, [2, 2, 4])
    out_sizes = list(reversed(_piece_sizes(C, [2, 2, 4])))

    nc = bacc.Bacc("TRN2", target_bir_lowering=False, debug=not axon_active(),
                   num_devices=N_CORES)
    dt = mybir.dt.float32
    f32r = mybir.dt.float32r
    x_d = nc.dram_tensor("x", [CI * L, b_core], f32r, kind="ExternalInput")
    ha_d = nc.dram_tensor("h_a", [L, L], f32r, kind="ExternalInput")
    hb_d = nc.dram_tensor("h_b", [L, L], f32r, kind="ExternalInput")
    y_d = nc.dram_tensor("y", [C * L, b_core], dt, kind="ExternalOutput")

    xv = x_d.rearrange("(c p) b -> p c b", p=L)     # [128, CI, b]
    yv = y_d.rearrange("(c p) b -> p c b", p=L)     # [128, C, b]

    with tile.TileContext(nc) as tc:
        with (
            tc.tile_pool(name="consts", bufs=1) as consts,
            tc.tile_pool(name="xin", bufs=1) as xpool,
            tc.tile_pool(name="yout", bufs=1) as ypool,
            tc.tile_pool(name="bank", bufs=6, space="PSUM") as bankpool,
        ):
            # H consts ride SWDGE (gpsimd) — a separate descriptor path, so
            # they don't delay the first x piece on sync's HWDGE
            ha_s = consts.tile([L, L], f32r)
            nc.gpsimd.dma_start(ha_s[:], ha_d[:])
            hb_s = consts.tile([L, L], f32r)
            nc.gpsimd.dma_start(hb_s[:], hb_d[:])

            # small leading input pieces -> first matmul starts early;
            # small trailing output pieces -> last store lands quickly
            xin = {}          # input chunk index -> (piece tile, offset)
            c0 = 0
            for pi, w in enumerate(in_sizes):
                t_ = xpool.tile([L, w, b_core], f32r, tag=f"xin{pi}")
                nc.sync.dma_start(t_[:], xv[:, c0:c0 + w, :])
                for k in range(w):
                    xin[c0 + k] = (t_, k)
                c0 += w

            def in_chunk(ci):
                t_, k = xin[ci]
                return t_[:, k, :]

            c0 = 0
            for po, w in enumerate(out_sizes):
                yt = ypool.tile([L, w, b_core], dt, tag=f"yout{po}")
                for c in range(c0, c0 + w):
                    bank = bankpool.tile([L, b_core], dt)
                    # output chunk c: H_a on input chunk c+1, H_b on chunk c
                    nc.tensor.matmul(bank[:], ha_s[:], in_chunk(c + 1),
                                     start=True, stop=False)
                    nc.tensor.matmul(bank[:], hb_s[:], in_chunk(c),
                                     start=False, stop=True)
                    nc.vector.tensor_copy(yt[:, c - c0, :], bank[:])
                nc.sync.dma_start(yv[:, c0:c0 + w, :], yt[:])
                c0 += w

    nc.compile()
    return nc


_CACHE = {}


def _get_module(b_core, t_core):
    key = (b_core, t_core)
    if key not in _CACHE:
        _CACHE[key] = _build_module(b_core, t_core)
    return _CACHE[key]


def _host_prep(g, r, m_hp, m_bp, m_lp):
    b, a = _filter_coeffs(float(np.asarray(g).reshape(-1)[0]),
                          float(np.asarray(r).reshape(-1)[0]),
                          float(np.asarray(m_hp).reshape(-1)[0]),
                          float(np.asarray(m_bp).reshape(-1)[0]),
                          float(np.asarray(m_lp).reshape(-1)[0]))
    h = _impulse_response(b, a, 4 * MAX_K)
    K = _choose_taps(h)
    if K > MAX_K:
        raise ValueError(f"need {K} taps > {MAX_K} supported")
    return _toeplitz_pieces(h, K)


def kernel(x, g, r, m_hp, m_bp, m_lp):
    x = np.asarray(x, dtype=np.float32)
    B, T = x.shape
    assert B % BS == 0 and T % TH == 0
    b_core, t_core = B // BS, T // TH
    assert t_core % L == 0

    h_a, h_b = _host_prep(g, r, m_hp, m_bp, m_lp)
    nc = _get_module(b_core, t_core)

    in_maps = []
    for bs in range(BS):
        xt = np.ascontiguousarray(x[bs * b_core:(bs + 1) * b_core, :].T)
        for th in range(TH):
            x_in = np.empty((t_core + L, b_core), dtype=np.float32)
            t0 = th * t_core
            if th == 0:
                x_in[:L] = 0.0
            else:
                x_in[:L] = xt[t0 - L:t0]
            x_in[L:] = xt[t0:t0 + t_core]
            in_maps.append({"x": x_in, "h_a": h_a, "h_b": h_b})

    res = run_bass_kernel_spmd(nc, in_maps, core_ids=list(range(N_CORES)))

    y = np.empty((B, T), dtype=np.float32)
    for bs in range(BS):
        for th in range(TH):
            yc = res.results[bs * TH + th]["y"]
            y[bs * b_core:(bs + 1) * b_core,
              th * t_core:(th + 1) * t_core] = yc.T
    return y


# revision 36
# speedup vs baseline: 1.0015x; 1.0015x over previous
"""Trainium2 Bass kernel for nn_DSVF (direct-form-I biquad IIR over time).

Algorithm
---------
The biquad is a stable IIR (poles well inside the unit circle for any
parameters setup_inputs-style params produce), so its impulse response decays
geometrically.  We truncate it at K taps (K chosen at runtime from the actual
coefficients so the truncated tail is < 1e-9; K is ~32 here) and compute the
filter as a causal FIR convolution — numerically indistinguishable from the
reference (the truncation error is far below the reference's own fp32
accumulation noise).

Kernel structure
----------------
The convolution is a Toeplitz matmul over 128-sample time chunks:

    yT[c] = H_a^T @ xT[c] + H_b^T @ xT[c-1]

where xT/yT are time-major [time, batch] chunks ([128, 512] on chip:
time-within-chunk on partitions, batch on the free axis), and
H_a[s, j] = h[j - s], H_b[s, j] = h[128 + j - s] are [128, 128] Toeplitz
pieces of the impulse response.  Each output chunk is exactly two tensor-
engine matmuls accumulating in one PSUM bank, with the moving operand at the
fp32 maximum free size (512).  No on-chip transposes: the host hands each
core its shard already in [time, batch] layout (part of the sharding step),
and un-transposes the [time, batch] result on the way back.  Matmuls run in
float32r (one half-clock PE pass instead of fp32's two).

Sharding
--------
8 cores = 4 batch shards x 2 time shards.  Batch is pure data parallel; the
time split is exact because the truncated FIR only needs a 128-sample halo,
which each second-half core receives (first-half cores get a zero halo).
The 5 scalar filter parameters are folded into H_a/H_b on the host.
"""
import math

import numpy as np

import concourse.bacc as bacc
import concourse.mybir as mybir
import concourse.tile as tile
from concourse.bass_utils import axon_active, run_bass_kernel_spmd

N_CORES = 8
BS, TH = 4, 2          # batch shards x time shards
L = 128                # time chunk (partition count)
MAX_K = L + 1          # max taps the 2-matmul window supports
PIECE = 8              # chunks per DMA piece


def _filter_coeffs(g, r, m_hp, m_bp, m_lp):
    """float32 mirror of the reference coefficient computation."""
    f32 = np.float32
    g = f32(g)
    r = f32(r)
    m_hp, m_bp, m_lp = f32(m_hp), f32(m_bp), f32(m_lp)
    gg = np.tan(f32(math.pi) * (f32(1) / (f32(1) + np.exp(-g))) / f32(2))
    rr = np.log1p(np.exp(r)).astype(f32)
    g2 = gg * gg
    b = np.array(
        [g2 * m_lp + gg * m_bp + m_hp,
         2 * g2 * m_lp - 2 * m_hp,
         g2 * m_lp - gg * m_bp + m_hp],
        dtype=f32,
    )
    a = np.array(
        [g2 + 2 * rr * gg + 1,
         2 * g2 - 2,
         g2 - 2 * rr * gg + 1],
        dtype=f32,
    )
    a = (a / a[0]).astype(f32)
    # reference divides b by the already-normalized a[0] == 1: b unchanged
    return b.astype(np.float64), a.astype(np.float64)


def _impulse_response(b, a, n):
    h = np.zeros(n, dtype=np.float64)
    for t in range(n):
        acc = b[0] if t == 0 else 0.0
        if t == 1:
            acc += b[1]
        if t == 2:
            acc += b[2]
        if t >= 1:
            acc -= a[1] * h[t - 1]
        if t >= 2:
            acc -= a[2] * h[t - 2]
        h[t] = acc
    return h


def _choose_taps(h):
    tail = np.cumsum(np.abs(h[::-1]))[::-1]
    for k in range(3, len(h)):
        if tail[k] < 1e-9:
            return k
    raise ValueError(
        f"impulse response decays too slowly for the FIR formulation "
        f"(tail at {len(h)} taps = {tail[-1]:.3e})"
    )


def _toeplitz_pieces(h, K):
    h_a = np.zeros((L, L), dtype=np.float32)
    h_b = np.zeros((L, L), dtype=np.float32)
    for s in range(L):
        for j in range(L):
            k = j - s
            if 0 <= k < K:
                h_a[s, j] = h[k]
            k = L + j - s
            if 0 <= k < K:
                h_b[s, j] = h[k]
    return h_a, h_b


def _piece_sizes(n, first_small):
    """Split n chunks into DMA pieces; small leading pieces so compute can
    start early (and small trailing output pieces so the last store is
    quick)."""
    sizes = []
    remaining = n
    for w in first_small:
        if remaining <= 0:
            break
        w = min(w, remaining)
        sizes.append(w)
        remaining -= w
    while remaining > 0:
        w = min(PIECE, remaining)
        sizes.append(w)
        remaining -= w
    return sizes


def _build_module(b_core, t_core):
    """Per-core SPMD program: FIR chunks via two stationary-H matmuls each."""
    assert b_core <= 512 and t_core % L == 0
    C = t_core // L                     # output chunks per core
    CI = C + 1                          # input chunks incl. 1 halo chunk

    in_sizes = _piece_sizes(CI, [2, 2, 4])
    out_sizes = list(reversed(_piece_sizes(C, [2, 2, 4])))

    nc = bacc.Bacc("TRN2", target_bir_lowering=False, debug=not axon_active(),
                   num_devices=N_CORES)
    dt = mybir.dt.float32
    f32r = mybir.dt.float32r
    x_d = nc.dram_tensor("x", [CI * L, b_core], f32r, kind="ExternalInput")
    ha_d = nc.dram_tensor("h_a", [L, L], f32r, kind="ExternalInput")
    hb_d = nc.dram_tensor("h_b", [L, L], f32r, kind="ExternalInput")
    y_d = nc.dram_tensor("y", [C * L, b_core], dt, kind="ExternalOutput")

    xv = x_d.rearrange("(c p) b -> p c b", p=L)     # [128, CI, b]
    yv = y_d.rearrange("(c p) b -> p c b", p=L)     # [128, C, b]

    with tile.TileContext(nc) as tc:
        with (
            tc.tile_pool(name="consts", bufs=1) as consts,
            tc.tile_pool(name="xin", bufs=1) as xpool,
            tc.tile_pool(name="yout", bufs=1) as ypool,
            tc.tile_pool(name="bank", bufs=4, space="PSUM") as bankpool,
        ):
            # H consts ride SWDGE (gpsimd) — a separate descriptor path, so
            # they don't delay the first x piece on sync's HWDGE
            ha_s = consts.tile([L, L], f32r)
            nc.gpsimd.dma_start(ha_s[:], ha_d[:])
            hb_s = consts.tile([L, L], f32r)
            nc.gpsimd.dma_start(hb_s[:], hb_d[:])

            # small leading input pieces -> first matmul starts early;
            # small trailing output pieces -> last store lands quickly
            xin = {}          # input chunk index -> (piece tile, offset)
            c0 = 0
            for pi, w in enumerate(in_sizes):
                t_ = xpool.tile([L, w, b_core], f32r, tag=f"xin{pi}")
                nc.sync.dma_start(t_[:], xv[:, c0:c0 + w, :])
                for k in range(w):
                    xin[c0 + k] = (t_, k)
                c0 += w

            def in_chunk(ci):
                t_, k = xin[ci]
                return t_[:, k, :]

            c0 = 0
            for po, w in enumerate(out_sizes):
                yt = ypool.tile([L, w, b_core], dt, tag=f"yout{po}")
                # pair chunks so the same stationary H is issued back-to-back
                # (weight-load pull-ahead works better without alternation)
                for c in range(c0, c0 + w, 2):
                    pw = min(2, c0 + w - c)
                    banks = [bankpool.tile([L, b_core], dt, name="bank",
                                           tag="bank") for _ in range(pw)]
                    for o in range(pw):
                        nc.tensor.matmul(banks[o][:], ha_s[:],
                                         in_chunk(c + o + 1),
                                         start=True, stop=False)
                    for o in range(pw):
                        nc.tensor.matmul(banks[o][:], hb_s[:],
                                         in_chunk(c + o),
                                         start=False, stop=True)
                    for o in range(pw):
                        nc.vector.tensor_copy(yt[:, c + o - c0, :], banks[o][:])
                nc.sync.dma_start(yv[:, c0:c0 + w, :], yt[:])
                c0 += w

    nc.compile()
    return nc


_CACHE = {}


def _get_module(b_core, t_core):
    key = (b_core, t_core)
    if key not in _CACHE:
        _CACHE[key] = _build_module(b_core, t_core)
    return _CACHE[key]


def _host_prep(g, r, m_hp, m_bp, m_lp):
    b, a = _filter_coeffs(float(np.asarray(g).reshape(-1)[0]),
                          float(np.asarray(r).reshape(-1)[0]),
                          float(np.asarray(m_hp).reshape(-1)[0]),
                          float(np.asarray(m_bp).reshape(-1)[0]),
                          float(np.asarray(m_lp).reshape(-1)[0]))
    h = _impulse_response(b, a, 4 * MAX_K)
    K = _choose_taps(h)
    if K > MAX_K:
        raise ValueError(f"need {K} taps > {MAX_K} supported")
    return _toeplitz_pieces(h, K)


def kernel(x, g, r, m_hp, m_bp, m_lp):
    x = np.asarray(x, dtype=np.float32)
    B, T = x.shape
    assert B % BS == 0 and T % TH == 0
    b_core, t_core = B // BS, T // TH
    assert t_core % L == 0

    h_a, h_b = _host_prep(g, r, m_hp, m_bp, m_lp)
    nc = _get_module(b_core, t_core)

    in_maps = []
    for bs in range(BS):
        xt = np.ascontiguousarray(x[bs * b_core:(bs + 1) * b_core, :].T)
        for th in range(TH):
            x_in = np.empty((t_core + L, b_core), dtype=np.float32)
            t0 = th * t_core
            if th == 0:
                x_in[:L] = 0.0
            else:
                x_in[:L] = xt[t0 - L:t0]
            x_in[L:] = xt[t0:t0 + t_core]
            in_maps.append({"x": x_in, "h_a": h_a, "h_b": h_b})

    res = run_bass_kernel_spmd(nc, in_maps, core_ids=list(range(N_CORES)))

    y = np.empty((B, T), dtype=np.float32)
    for bs in range(BS):
        for th in range(TH):
            yc = res.results[bs * TH + th]["y"]
            y[bs * b_core:(bs + 1) * b_core,
              th * t_core:(th + 1) * t_core] = yc.T
    return y


# revision 37
# speedup vs baseline: 1.0122x; 1.0106x over previous
"""Trainium2 Bass kernel for nn_DSVF (direct-form-I biquad IIR over time).

Algorithm
---------
The biquad is a stable IIR (poles well inside the unit circle for any
parameters setup_inputs-style params produce), so its impulse response decays
geometrically.  We truncate it at K taps (K chosen at runtime from the actual
coefficients so the truncated tail is < 1e-9; K is ~32 here) and compute the
filter as a causal FIR convolution — numerically indistinguishable from the
reference (the truncation error is far below the reference's own fp32
accumulation noise).

Kernel structure
----------------
The convolution is a Toeplitz matmul over 128-sample time chunks:

    yT[c] = H_a^T @ xT[c] + H_b^T @ xT[c-1]

where xT/yT are time-major [time, batch] chunks ([128, 512] on chip:
time-within-chunk on partitions, batch on the free axis), and
H_a[s, j] = h[j - s], H_b[s, j] = h[128 + j - s] are [128, 128] Toeplitz
pieces of the impulse response.  Each output chunk is exactly two tensor-
engine matmuls accumulating in one PSUM bank, with the moving operand at the
fp32 maximum free size (512).  No on-chip transposes: the host hands each
core its shard already in [time, batch] layout (part of the sharding step),
and un-transposes the [time, batch] result on the way back.  Matmuls run in
float32r (one half-clock PE pass instead of fp32's two).

Sharding
--------
8 cores = 4 batch shards x 2 time shards.  Batch is pure data parallel; the
time split is exact because the truncated FIR only needs a 128-sample halo,
which each second-half core receives (first-half cores get a zero halo).
The 5 scalar filter parameters are folded into H_a/H_b on the host.
"""
import math

import numpy as np

import concourse.bacc as bacc
import concourse.mybir as mybir
import concourse.tile as tile
from concourse.bass_utils import axon_active, run_bass_kernel_spmd

N_CORES = 8
BS, TH = 4, 2          # batch shards x time shards
L = 128                # time chunk (partition count)
MAX_K = L + 1          # max taps the 2-matmul window supports
PIECE = 8              # chunks per DMA piece


def _filter_coeffs(g, r, m_hp, m_bp, m_lp):
    """float32 mirror of the reference coefficient computation."""
    f32 = np.float32
    g = f32(g)
    r = f32(r)
    m_hp, m_bp, m_lp = f32(m_hp), f32(m_bp), f32(m_lp)
    gg = np.tan(f32(math.pi) * (f32(1) / (f32(1) + np.exp(-g))) / f32(2))
    rr = np.log1p(np.exp(r)).astype(f32)
    g2 = gg * gg
    b = np.array(
        [g2 * m_lp + gg * m_bp + m_hp,
         2 * g2 * m_lp - 2 * m_hp,
         g2 * m_lp - gg * m_bp + m_hp],
        dtype=f32,
    )
    a = np.array(
        [g2 + 2 * rr * gg + 1,
         2 * g2 - 2,
         g2 - 2 * rr * gg + 1],
        dtype=f32,
    )
    a = (a / a[0]).astype(f32)
    # reference divides b by the already-normalized a[0] == 1: b unchanged
    return b.astype(np.float64), a.astype(np.float64)


def _impulse_response(b, a, n):
    h = np.zeros(n, dtype=np.float64)
    for t in range(n):
        acc = b[0] if t == 0 else 0.0
        if t == 1:
            acc += b[1]
        if t == 2:
            acc += b[2]
        if t >= 1:
            acc -= a[1] * h[t - 1]
        if t >= 2:
            acc -= a[2] * h[t - 2]
        h[t] = acc
    return h


def _choose_taps(h):
    tail = np.cumsum(np.abs(h[::-1]))[::-1]
    for k in range(3, len(h)):
        if tail[k] < 1e-9:
            return k
    raise ValueError(
        f"impulse response decays too slowly for the FIR formulation "
        f"(tail at {len(h)} taps = {tail[-1]:.3e})"
    )


def _toeplitz_pieces(h, K):
    h_a = np.zeros((L, L), dtype=np.float32)
    h_b = np.zeros((L, L), dtype=np.float32)
    for s in range(L):
        for j in range(L):
            k = j - s
            if 0 <= k < K:
                h_a[s, j] = h[k]
            k = L + j - s
            if 0 <= k < K:
                h_b[s, j] = h[k]
    return h_a, h_b


def _piece_sizes(n, first_small):
    """Split n chunks into DMA pieces; small leading pieces so compute can
    start early (and small trailing output pieces so the last store is
    quick)."""
    sizes = []
    remaining = n
    for w in first_small:
        if remaining <= 0:
            break
        w = min(w, remaining)
        sizes.append(w)
        remaining -= w
    while remaining > 0:
        w = min(PIECE, remaining)
        sizes.append(w)
        remaining -= w
    return sizes


def _build_module(b_core, t_core):
    """Per-core SPMD program: FIR chunks via two stationary-H matmuls each."""
    assert b_core <= 512 and t_core % L == 0
    C = t_core // L                     # output chunks per core
    CI = C + 1                          # input chunks incl. 1 halo chunk

    in_sizes = _piece_sizes(CI, [2, 2, 4])
    out_sizes = list(reversed(_piece_sizes(C, [2, 2, 4])))

    nc = bacc.Bacc("TRN2", target_bir_lowering=False, debug=not axon_active(),
                   num_devices=N_CORES)
    dt = mybir.dt.float32
    f32r = mybir.dt.float32r
    x_d = nc.dram_tensor("x", [CI * L, b_core], f32r, kind="ExternalInput")
    ha_d = nc.dram_tensor("h_a", [L, L], f32r, kind="ExternalInput")
    hb_d = nc.dram_tensor("h_b", [L, L], f32r, kind="ExternalInput")
    y_d = nc.dram_tensor("y", [C * L, b_core], dt, kind="ExternalOutput")

    xv = x_d.rearrange("(c p) b -> p c b", p=L)     # [128, CI, b]
    yv = y_d.rearrange("(c p) b -> p c b", p=L)     # [128, C, b]

    with tile.TileContext(nc) as tc:
        with (
            tc.tile_pool(name="consts", bufs=1) as consts,
            tc.tile_pool(name="xin", bufs=1) as xpool,
            tc.tile_pool(name="yout", bufs=1) as ypool,
            tc.tile_pool(name="bank", bufs=4, space="PSUM") as bankpool,
        ):
            # H consts ride SWDGE (gpsimd) — a separate descriptor path, so
            # they don't delay the first x piece on sync's HWDGE
            ha_s = consts.tile([L, L], f32r)
            nc.gpsimd.dma_start(ha_s[:], ha_d[:])
            hb_s = consts.tile([L, L], f32r)
            nc.gpsimd.dma_start(hb_s[:], hb_d[:])

            # small leading input pieces -> first matmul starts early;
            # small trailing output pieces -> last store lands quickly
            xin = {}          # input chunk index -> (piece tile, offset)
            c0 = 0
            for pi, w in enumerate(in_sizes):
                t_ = xpool.tile([L, w, b_core], f32r, tag=f"xin{pi}")
                nc.sync.dma_start(t_[:], xv[:, c0:c0 + w, :])
                for k in range(w):
                    xin[c0 + k] = (t_, k)
                c0 += w

            def in_chunk(ci):
                t_, k = xin[ci]
                return t_[:, k, :]

            c0 = 0
            for po, w in enumerate(out_sizes):
                yt = ypool.tile([L, w, b_core], dt, tag=f"yout{po}")
                for c in range(c0, c0 + w):
                    bank = bankpool.tile([L, b_core], dt)
                    # output chunk c: H_a on input chunk c+1, H_b on chunk c
                    nc.tensor.matmul(bank[:], ha_s[:], in_chunk(c + 1),
                                     start=True, stop=False)
                    nc.tensor.matmul(bank[:], hb_s[:], in_chunk(c),
                                     start=False, stop=True)
                    nc.vector.tensor_copy(yt[:, c - c0, :], bank[:])
                nc.sync.dma_start(yv[:, c0:c0 + w, :], yt[:])
                c0 += w

    nc.compile()
    return nc


_CACHE = {}


def _get_module(b_core, t_core):
    key = (b_core, t_core)
    if key not in _CACHE:
        _CACHE[key] = _build_module(b_core, t_core)
    return _CACHE[key]


def _host_prep(g, r, m_hp, m_bp, m_lp):
    b, a = _filter_coeffs(float(np.asarray(g).reshape(-1)[0]),
                          float(np.asarray(r).reshape(-1)[0]),
                          float(np.asarray(m_hp).reshape(-1)[0]),
                          float(np.asarray(m_bp).reshape(-1)[0]),
                          float(np.asarray(m_lp).reshape(-1)[0]))
    h = _impulse_response(b, a, 4 * MAX_K)
    K = _choose_taps(h)
    if K > MAX_K:
        raise ValueError(f"need {K} taps > {MAX_K} supported")
    return _toeplitz_pieces(h, K)


def kernel(x, g, r, m_hp, m_bp, m_lp):
    x = np.asarray(x, dtype=np.float32)
    B, T = x.shape
    assert B % BS == 0 and T % TH == 0
    b_core, t_core = B // BS, T // TH
    assert t_core % L == 0

    h_a, h_b = _host_prep(g, r, m_hp, m_bp, m_lp)
    nc = _get_module(b_core, t_core)

    in_maps = []
    for bs in range(BS):
        xt = np.ascontiguousarray(x[bs * b_core:(bs + 1) * b_core, :].T)
        for th in range(TH):
            x_in = np.empty((t_core + L, b_core), dtype=np.float32)
            t0 = th * t_core
            if th == 0:
                x_in[:L] = 0.0
            else:
                x_in[:L] = xt[t0 - L:t0]
            x_in[L:] = xt[t0:t0 + t_core]
            in_maps.append({"x": x_in, "h_a": h_a, "h_b": h_b})

    res = run_bass_kernel_spmd(nc, in_maps, core_ids=list(range(N_CORES)))

    y = np.empty((B, T), dtype=np.float32)
    for bs in range(BS):
        for th in range(TH):
            yc = res.results[bs * TH + th]["y"]
            y[bs * b_core:(bs + 1) * b_core,
              th * t_core:(th + 1) * t_core] = yc.T
    return y


# revision 39
# speedup vs baseline: 1.0325x; 1.0201x over previous
"""Trainium2 Bass kernel for nn_DSVF (direct-form-I biquad IIR over time).

Algorithm
---------
The biquad is a stable IIR (poles well inside the unit circle for any
parameters setup_inputs-style params produce), so its impulse response decays
geometrically.  We truncate it at K taps (K chosen at runtime from the actual
coefficients so the truncated tail is < 1e-9; K is ~32 here) and compute the
filter as a causal FIR convolution — numerically indistinguishable from the
reference (the truncation error is far below the reference's own fp32
accumulation noise).

Kernel structure
----------------
The convolution is a Toeplitz matmul over 128-sample time chunks:

    yT[c] = H_a^T @ xT[c] + H_b^T @ xT[c-1]

where xT/yT are time-major [time, batch] chunks ([128, 512] on chip:
time-within-chunk on partitions, batch on the free axis), and
H_a[s, j] = h[j - s], H_b[s, j] = h[128 + j - s] are [128, 128] Toeplitz
pieces of the impulse response.  Each output chunk is exactly two tensor-
engine matmuls accumulating in one PSUM bank, with the moving operand at the
fp32 maximum free size (512).  No on-chip transposes: the host hands each
core its shard already in [time, batch] layout (part of the sharding step),
and un-transposes the [time, batch] result on the way back.  Matmuls run in
float32r (one half-clock PE pass instead of fp32's two).

Sharding
--------
8 cores = 4 batch shards x 2 time shards.  Batch is pure data parallel; the
time split is exact because the truncated FIR only needs a 128-sample halo,
which each second-half core receives (first-half cores get a zero halo).
The 5 scalar filter parameters are folded into H_a/H_b on the host.
"""
import contextlib
import math

import numpy as np

import concourse.bacc as bacc
import concourse.mybir as mybir
from concourse.bass_utils import axon_active, run_bass_kernel_spmd

N_CORES = 8
BS, TH = 4, 2          # batch shards x time shards
L = 128                # time chunk (partition count)
MAX_K = L + 1          # max taps the 2-matmul window supports
PIECE = 8              # chunks per DMA piece


def _filter_coeffs(g, r, m_hp, m_bp, m_lp):
    """float32 mirror of the reference coefficient computation."""
    f32 = np.float32
    g = f32(g)
    r = f32(r)
    m_hp, m_bp, m_lp = f32(m_hp), f32(m_bp), f32(m_lp)
    gg = np.tan(f32(math.pi) * (f32(1) / (f32(1) + np.exp(-g))) / f32(2))
    rr = np.log1p(np.exp(r)).astype(f32)
    g2 = gg * gg
    b = np.array(
        [g2 * m_lp + gg * m_bp + m_hp,
         2 * g2 * m_lp - 2 * m_hp,
         g2 * m_lp - gg * m_bp + m_hp],
        dtype=f32,
    )
    a = np.array(
        [g2 + 2 * rr * gg + 1,
         2 * g2 - 2,
         g2 - 2 * rr * gg + 1],
        dtype=f32,
    )
    a = (a / a[0]).astype(f32)
    # reference divides b by the already-normalized a[0] == 1: b unchanged
    return b.astype(np.float64), a.astype(np.float64)


def _impulse_response(b, a, n):
    h = np.zeros(n, dtype=np.float64)
    for t in range(n):
        acc = b[0] if t == 0 else 0.0
        if t == 1:
            acc += b[1]
        if t == 2:
            acc += b[2]
        if t >= 1:
            acc -= a[1] * h[t - 1]
        if t >= 2:
            acc -= a[2] * h[t - 2]
        h[t] = acc
    return h


def _choose_taps(h):
    tail = np.cumsum(np.abs(h[::-1]))[::-1]
    for k in range(3, len(h)):
        if tail[k] < 1e-9:
            return k
    raise ValueError(
        f"impulse response decays too slowly for the FIR formulation "
        f"(tail at {len(h)} taps = {tail[-1]:.3e})"
    )


def _toeplitz_pieces(h, K):
    h_a = np.zeros((L, L), dtype=np.float32)
    h_b = np.zeros((L, L), dtype=np.float32)
    for s in range(L):
        for j in range(L):
            k = j - s
            if 0 <= k < K:
                h_a[s, j] = h[k]
            k = L + j - s
            if 0 <= k < K:
                h_b[s, j] = h[k]
    return h_a, h_b


def _piece_sizes(n, first_small):
    """Split n chunks into DMA pieces; small leading pieces so compute can
    start early (and small trailing output pieces so the last store is
    quick)."""
    sizes = []
    remaining = n
    for w in first_small:
        if remaining <= 0:
            break
        w = min(w, remaining)
        sizes.append(w)
        remaining -= w
    while remaining > 0:
        w = min(PIECE, remaining)
        sizes.append(w)
        remaining -= w
    return sizes


def _build_module(b_core, t_core):
    """Per-core SPMD program: FIR chunks via two stationary-H matmuls each.

    Raw bacc with hand-rolled semaphores (a Tile version of the same dataflow
    measures ~6-7us slower purely from its ~57-semaphore preamble/drain
    choreography).  Pipeline: sync streams x pieces in and y pieces out;
    the tensor engine runs two matmuls per output chunk into one of NSLOT
    rotating PSUM banks; the vector engine copies each finished bank into the
    y staging buffer.  PE waits on the copy sem before reusing a bank (WAR +
    the PE-write/DVE-read same-bank hazard), copies wait on the matmul sem,
    stores wait on the copy sem."""
    assert b_core <= 512 and t_core % L == 0
    C = t_core // L                     # output chunks per core
    CI = C + 1                          # input chunks incl. 1 halo chunk

    in_sizes = _piece_sizes(CI, [2, 2, 4])
    out_sizes = list(reversed(_piece_sizes(C, [2, 2, 4])))
    n_out = len(out_sizes)
    in_start = np.cumsum([0] + in_sizes)
    out_start = np.cumsum([0] + out_sizes)

    def piece_of(ci):
        return int(np.searchsorted(in_start, ci, side="right") - 1)

    nc = bacc.Bacc("TRN2", target_bir_lowering=False, debug=not axon_active(),
                   num_devices=N_CORES)
    dt = mybir.dt.float32
    f32r = mybir.dt.float32r
    x_d = nc.dram_tensor("x", [CI * L, b_core], f32r, kind="ExternalInput")
    ha_d = nc.dram_tensor("h_a", [L, L], f32r, kind="ExternalInput")
    hb_d = nc.dram_tensor("h_b", [L, L], f32r, kind="ExternalInput")
    y_d = nc.dram_tensor("y", [C * L, b_core], dt, kind="ExternalOutput")
    xv = x_d.rearrange("(c p) b -> p c b", p=L)     # [128, CI, b]
    yv = y_d.rearrange("(c p) b -> p c b", p=L)     # [128, C, b]

    NSLOT = 4
    with contextlib.ExitStack() as ctx:
        ha_s = ctx.enter_context(nc.sbuf_tensor("ha_s", [L, L], f32r))
        hb_s = ctx.enter_context(nc.sbuf_tensor("hb_s", [L, L], f32r))
        x_s = ctx.enter_context(nc.sbuf_tensor("x_s", [L, CI * b_core], f32r))
        y_s = ctx.enter_context(nc.sbuf_tensor("y_s", [L, C * b_core], dt))
        banks = [ctx.enter_context(nc.psum_tensor(f"bank{i}", [L, b_core], dt))
                 for i in range(NSLOT)]
        c_sem = ctx.enter_context(nc.semaphore("c_sem"))
        p_sems = [ctx.enter_context(nc.semaphore(f"p{i}"))
                  for i in range(len(in_sizes))]
        mm_sem = ctx.enter_context(nc.semaphore("mm_sem"))
        cp_sem = ctx.enter_context(nc.semaphore("cp_sem"))
        o_sem = ctx.enter_context(nc.semaphore("o_sem"))
        block = ctx.enter_context(nc.Block())

        def xchunk(ci):
            return x_s[:, ci * b_core:(ci + 1) * b_core]

        @block.sync
        def _(sync):
            sync.dma_start(ha_s[:], ha_d[:]).then_inc(c_sem, 16)
            sync.dma_start(hb_s[:], hb_d[:]).then_inc(c_sem, 16)
            for pi, w in enumerate(in_sizes):
                c0 = int(in_start[pi])
                sync.dma_start(
                    x_s[:, c0 * b_core:(c0 + w) * b_core].rearrange(
                        "p (c b) -> p c b", c=w),
                    xv[:, c0:c0 + w, :],
                ).then_inc(p_sems[pi], 16)
            for po, w in enumerate(out_sizes):
                c0 = int(out_start[po])
                sync.wait_ge(cp_sem, c0 + w)
                sync.dma_start(
                    yv[:, c0:c0 + w, :],
                    y_s[:, c0 * b_core:(c0 + w) * b_core].rearrange(
                        "p (c b) -> p c b", c=w),
                ).then_inc(o_sem, 16)
            sync.wait_ge(o_sem, 16 * n_out)

        @block.tensor
        def _(tensor):
            tensor.wait_ge(c_sem, 32)
            for c in range(C):
                if c >= NSLOT:
                    # bank recycle: previous occupant must be copied out
                    tensor.wait_ge(cp_sem, c - (NSLOT - 1))
                need = piece_of(c + 1)
                tensor.wait_ge(p_sems[need], 16)
                if need > 0:
                    tensor.wait_ge(p_sems[need - 1], 16)
                bank = banks[c % NSLOT]
                # output chunk c: H_a on input chunk c+1, H_b on chunk c
                nc.tensor.matmul(bank[:], ha_s[:], xchunk(c + 1),
                                 start=True, stop=False)
                nc.tensor.matmul(bank[:], hb_s[:], xchunk(c),
                                 start=False, stop=True).then_inc(mm_sem, 1)

        @block.vector
        def _(vector):
            for c in range(C):
                vector.wait_ge(mm_sem, c + 1)
                nc.vector.tensor_copy(
                    y_s[:, c * b_core:(c + 1) * b_core],
                    banks[c % NSLOT][:],
                ).then_inc(cp_sem, 1)

    nc.compile()
    return nc


_CACHE = {}


def _get_module(b_core, t_core):
    key = (b_core, t_core)
    if key not in _CACHE:
        _CACHE[key] = _build_module(b_core, t_core)
    return _CACHE[key]


def _host_prep(g, r, m_hp, m_bp, m_lp):
    b, a = _filter_coeffs(float(np.asarray(g).reshape(-1)[0]),
                          float(np.asarray(r).reshape(-1)[0]),
                          float(np.asarray(m_hp).reshape(-1)[0]),
                          float(np.asarray(m_bp).reshape(-1)[0]),
                          float(np.asarray(m_lp).reshape(-1)[0]))
    h = _impulse_response(b, a, 4 * MAX_K)
    K = _choose_taps(h)
    if K > MAX_K:
        raise ValueError(f"need {K} taps > {MAX_K} supported")
    return _toeplitz_pieces(h, K)


def kernel(x, g, r, m_hp, m_bp, m_lp):
    x = np.asarray(x, dtype=np.float32)
    B, T = x.shape
    assert B % BS == 0 and T % TH == 0
    b_core, t_core = B // BS, T // TH
    assert t_core % L == 0

    h_a, h_b = _host_prep(g, r, m_hp, m_bp, m_lp)
    nc = _get_module(b_core, t_core)

    in_maps = []
    for bs in range(BS):
        xt = np.ascontiguousarray(x[bs * b_core:(bs + 1) * b_core, :].T)
        for th in range(TH):
            x_in = np.empty((t_core + L, b_core), dtype=np.float32)
            t0 = th * t_core
            if th == 0:
                x_in[:L] = 0.0
            else:
                x_in[:L] = xt[t0 - L:t0]
            x_in[L:] = xt[t0:t0 + t_core]
            in_maps.append({"x": x_in, "h_a": h_a, "h_b": h_b})

    res = run_bass_kernel_spmd(nc, in_maps, core_ids=list(range(N_CORES)))

    y = np.empty((B, T), dtype=np.float32)
    for bs in range(BS):
        for th in range(TH):
            yc = res.results[bs * TH + th]["y"]
            y[bs * b_core:(bs + 1) * b_core,
              th * t_core:(th + 1) * t_core] = yc.T
    return y


# revision 41
# speedup vs baseline: 1.1381x; 1.1022x over previous
"""Trainium2 Bass kernel for nn_DSVF (direct-form-I biquad IIR over time).

Algorithm
---------
The biquad is a stable IIR (poles well inside the unit circle for any
parameters setup_inputs-style params produce), so its impulse response decays
geometrically.  We truncate it at K taps (K chosen at runtime from the actual
coefficients so the truncated tail is < 1e-9; K is ~32 here) and compute the
filter as a causal FIR convolution — numerically indistinguishable from the
reference (the truncation error is far below the reference's own fp32
accumulation noise).

Kernel structure
----------------
The convolution is a Toeplitz matmul over 128-sample time chunks:

    yT[c] = H_a^T @ xT[c] + H_b^T @ xT[c-1]

where xT/yT are time-major [time, batch] chunks ([128, 512] on chip:
time-within-chunk on partitions, batch on the free axis), and
H_a[s, j] = h[j - s], H_b[s, j] = h[128 + j - s] are [128, 128] Toeplitz
pieces of the impulse response.  Each output chunk is exactly two tensor-
engine matmuls accumulating in one PSUM bank, with the moving operand at the
fp32 maximum free size (512).  No on-chip transposes: the host hands each
core its shard already in [time, batch] layout (part of the sharding step),
and un-transposes the [time, batch] result on the way back.  Matmuls run in
float32r (one half-clock PE pass instead of fp32's two).

Sharding
--------
8 cores = 4 batch shards x 2 time shards.  Batch is pure data parallel; the
time split is exact because the truncated FIR only needs a 128-sample halo,
which each second-half core receives (first-half cores get a zero halo).
The 5 scalar filter parameters are folded into H_a/H_b on the host.
"""
import contextlib
import math

import numpy as np

import concourse.bacc as bacc
import concourse.mybir as mybir
from concourse.bass_utils import axon_active, run_bass_kernel_spmd

N_CORES = 8
BS, TH = 4, 2          # batch shards x time shards
L = 128                # time chunk (partition count)
MAX_K = L + 1          # max taps the 2-matmul window supports
PIECE = 8              # chunks per DMA piece


def _filter_coeffs(g, r, m_hp, m_bp, m_lp):
    """float32 mirror of the reference coefficient computation."""
    f32 = np.float32
    g = f32(g)
    r = f32(r)
    m_hp, m_bp, m_lp = f32(m_hp), f32(m_bp), f32(m_lp)
    gg = np.tan(f32(math.pi) * (f32(1) / (f32(1) + np.exp(-g))) / f32(2))
    rr = np.log1p(np.exp(r)).astype(f32)
    g2 = gg * gg
    b = np.array(
        [g2 * m_lp + gg * m_bp + m_hp,
         2 * g2 * m_lp - 2 * m_hp,
         g2 * m_lp - gg * m_bp + m_hp],
        dtype=f32,
    )
    a = np.array(
        [g2 + 2 * rr * gg + 1,
         2 * g2 - 2,
         g2 - 2 * rr * gg + 1],
        dtype=f32,
    )
    a = (a / a[0]).astype(f32)
    # reference divides b by the already-normalized a[0] == 1: b unchanged
    return b.astype(np.float64), a.astype(np.float64)


def _impulse_response(b, a, n):
    h = np.zeros(n, dtype=np.float64)
    for t in range(n):
        acc = b[0] if t == 0 else 0.0
        if t == 1:
            acc += b[1]
        if t == 2:
            acc += b[2]
        if t >= 1:
            acc -= a[1] * h[t - 1]
        if t >= 2:
            acc -= a[2] * h[t - 2]
        h[t] = acc
    return h


def _choose_taps(h):
    tail = np.cumsum(np.abs(h[::-1]))[::-1]
    for k in range(3, len(h)):
        if tail[k] < 1e-9:
            return k
    raise ValueError(
        f"impulse response decays too slowly for the FIR formulation "
        f"(tail at {len(h)} taps = {tail[-1]:.3e})"
    )


def _toeplitz_pieces(h, K):
    h_a = np.zeros((L, L), dtype=np.float32)
    h_b = np.zeros((L, L), dtype=np.float32)
    for s in range(L):
        for j in range(L):
            k = j - s
            if 0 <= k < K:
                h_a[s, j] = h[k]
            k = L + j - s
            if 0 <= k < K:
                h_b[s, j] = h[k]
    return h_a, h_b


def _piece_sizes(n, first_small):
    """Split n chunks into DMA pieces; small leading pieces so compute can
    start early (and small trailing output pieces so the last store is
    quick)."""
    sizes = []
    remaining = n
    for w in first_small:
        if remaining <= 0:
            break
        w = min(w, remaining)
        sizes.append(w)
        remaining -= w
    while remaining > 0:
        w = min(PIECE, remaining)
        sizes.append(w)
        remaining -= w
    return sizes


def _build_module(b_core, t_core):
    """Per-core SPMD program: FIR chunks via two stationary-H matmuls each.

    Raw bacc with hand-rolled semaphores (a Tile version of the same dataflow
    measures ~6-7us slower purely from its ~57-semaphore preamble/drain
    choreography).  Pipeline: sync streams x pieces in and y pieces out;
    the tensor engine runs two matmuls per output chunk into one of NSLOT
    rotating PSUM banks; the vector engine copies each finished bank into the
    y staging buffer.  PE waits on the copy sem before reusing a bank (WAR +
    the PE-write/DVE-read same-bank hazard), copies wait on the matmul sem,
    stores wait on the copy sem."""
    assert b_core <= 512 and t_core % L == 0
    C = t_core // L                     # output chunks per core
    CI = C + 1                          # input chunks incl. 1 halo chunk

    in_sizes = _piece_sizes(CI, [2, 2, 4])
    out_sizes = list(reversed(_piece_sizes(C, [2, 2, 4])))
    n_out = len(out_sizes)
    in_start = np.cumsum([0] + in_sizes)
    out_start = np.cumsum([0] + out_sizes)

    def piece_of(ci):
        return int(np.searchsorted(in_start, ci, side="right") - 1)

    nc = bacc.Bacc("TRN2", target_bir_lowering=False, debug=not axon_active(),
                   num_devices=N_CORES)
    dt = mybir.dt.float32
    f32r = mybir.dt.float32r
    x_d = nc.dram_tensor("x", [CI * L, b_core], f32r, kind="ExternalInput")
    ha_d = nc.dram_tensor("h_a", [L, L], f32r, kind="ExternalInput")
    hb_d = nc.dram_tensor("h_b", [L, L], f32r, kind="ExternalInput")
    y_d = nc.dram_tensor("y", [C * L, b_core], dt, kind="ExternalOutput")
    xv = x_d.rearrange("(c p) b -> p c b", p=L)     # [128, CI, b]
    yv = y_d.rearrange("(c p) b -> p c b", p=L)     # [128, C, b]

    NSLOT = 6
    with contextlib.ExitStack() as ctx:
        ha_s = ctx.enter_context(nc.sbuf_tensor("ha_s", [L, L], f32r))
        hb_s = ctx.enter_context(nc.sbuf_tensor("hb_s", [L, L], f32r))
        x_s = ctx.enter_context(nc.sbuf_tensor("x_s", [L, CI * b_core], f32r))
        y_s = ctx.enter_context(nc.sbuf_tensor("y_s", [L, C * b_core], dt))
        banks = [ctx.enter_context(nc.psum_tensor(f"bank{i}", [L, b_core], dt))
                 for i in range(NSLOT)]
        c_sem = ctx.enter_context(nc.semaphore("c_sem"))
        p_sems = [ctx.enter_context(nc.semaphore(f"p{i}"))
                  for i in range(len(in_sizes))]
        mm_sem = ctx.enter_context(nc.semaphore("mm_sem"))
        cp_sem = ctx.enter_context(nc.semaphore("cp_sem"))
        o_sem = ctx.enter_context(nc.semaphore("o_sem"))
        block = ctx.enter_context(nc.Block())

        def xchunk(ci):
            return x_s[:, ci * b_core:(ci + 1) * b_core]

        @block.sync
        def _(sync):
            sync.dma_start(ha_s[:], ha_d[:]).then_inc(c_sem, 16)
            sync.dma_start(hb_s[:], hb_d[:]).then_inc(c_sem, 16)
            for pi, w in enumerate(in_sizes):
                c0 = int(in_start[pi])
                sync.dma_start(
                    x_s[:, c0 * b_core:(c0 + w) * b_core].rearrange(
                        "p (c b) -> p c b", c=w),
                    xv[:, c0:c0 + w, :],
                ).then_inc(p_sems[pi], 16)
            for po, w in enumerate(out_sizes):
                c0 = int(out_start[po])
                sync.wait_ge(cp_sem, c0 + w)
                sync.dma_start(
                    yv[:, c0:c0 + w, :],
                    y_s[:, c0 * b_core:(c0 + w) * b_core].rearrange(
                        "p (c b) -> p c b", c=w),
                ).then_inc(o_sem, 16)
            sync.wait_ge(o_sem, 16 * n_out)

        @block.tensor
        def _(tensor):
            tensor.wait_ge(c_sem, 32)
            waited = set()
            for c in range(C):
                if c >= NSLOT:
                    # bank recycle: previous occupant must be copied out
                    tensor.wait_ge(cp_sem, c - (NSLOT - 1))
                # engine waits are cumulative: one wait per piece suffices
                for pi in (piece_of(c), piece_of(c + 1)):
                    if pi not in waited:
                        waited.add(pi)
                        tensor.wait_ge(p_sems[pi], 16)
                bank = banks[c % NSLOT]
                # output chunk c: H_a on input chunk c+1, H_b on chunk c
                nc.tensor.matmul(bank[:], ha_s[:], xchunk(c + 1),
                                 start=True, stop=False)
                nc.tensor.matmul(bank[:], hb_s[:], xchunk(c),
                                 start=False, stop=True).then_inc(mm_sem, 1)

        @block.vector
        def _(vector):
            for c in range(C):
                vector.wait_ge(mm_sem, c + 1)
                nc.vector.tensor_copy(
                    y_s[:, c * b_core:(c + 1) * b_core],
                    banks[c % NSLOT][:],
                ).then_inc(cp_sem, 1)

    nc.compile()
    return nc


_CACHE = {}


def _get_module(b_core, t_core):
    key = (b_core, t_core)
    if key not in _CACHE:
        _CACHE[key] = _build_module(b_core, t_core)
    return _CACHE[key]


def _host_prep(g, r, m_hp, m_bp, m_lp):
    b, a = _filter_coeffs(float(np.asarray(g).reshape(-1)[0]),
                          float(np.asarray(r).reshape(-1)[0]),
                          float(np.asarray(m_hp).reshape(-1)[0]),
                          float(np.asarray(m_bp).reshape(-1)[0]),
                          float(np.asarray(m_lp).reshape(-1)[0]))
    h = _impulse_response(b, a, 4 * MAX_K)
    K = _choose_taps(h)
    if K > MAX_K:
        raise ValueError(f"need {K} taps > {MAX_K} supported")
    return _toeplitz_pieces(h, K)


def kernel(x, g, r, m_hp, m_bp, m_lp):
    x = np.asarray(x, dtype=np.float32)
    B, T = x.shape
    assert B % BS == 0 and T % TH == 0
    b_core, t_core = B // BS, T // TH
    assert t_core % L == 0

    h_a, h_b = _host_prep(g, r, m_hp, m_bp, m_lp)
    nc = _get_module(b_core, t_core)

    in_maps = []
    for bs in range(BS):
        xt = np.ascontiguousarray(x[bs * b_core:(bs + 1) * b_core, :].T)
        for th in range(TH):
            x_in = np.empty((t_core + L, b_core), dtype=np.float32)
            t0 = th * t_core
            if th == 0:
                x_in[:L] = 0.0
            else:
                x_in[:L] = xt[t0 - L:t0]
            x_in[L:] = xt[t0:t0 + t_core]
            in_maps.append({"x": x_in, "h_a": h_a, "h_b": h_b})

    res = run_bass_kernel_spmd(nc, in_maps, core_ids=list(range(N_CORES)))

    y = np.empty((B, T), dtype=np.float32)
    for bs in range(BS):
        for th in range(TH):
            yc = res.results[bs * TH + th]["y"]
            y[bs * b_core:(bs + 1) * b_core,
              th * t_core:(th + 1) * t_core] = yc.T
    return y
